# revision 21
# baseline (speedup 1.0000x reference)
"""GGNN (Devign) message-passing kernel for 8 Trainium2 NeuronCores.

Strategy (graph-parallel, dst-sharded):
  - Nodes are sharded across 8 cores at graph boundaries (32 graphs/core) so
    max-pooling stays local. Node columns are graph-aligned (cross-core common
    per-graph stride) so pooling ranges are compile-time constants (SPMD).
  - Per step: each core computes m = h @ W for its shard (node-major, padded
    256-col bf16 rows), AllGathers m into a shared 8*SH-row table in DRAM,
    gathers the rows for its in-edges with bulk dma_gather (two int16-indexed
    views of the table, split by source row < / >= 4*SH), segment-sums them
    with log-depth DVE folds over a degree-sorted ELL layout, scatter-adds
    per-node partials into a local agg table (dma_scatter_add), DMA-transposes
    agg into feature-major SBUF, and runs the GRU with bf16 matmuls (f32 PSUM
    accum, biases folded in via a constant ones-row at K index 72 of block 1).
  - After 6 steps: mask junk columns, per-graph reduce_max pooling, ReLU,
    2x200 classifier matmul, sigmoid, output [2, 32] per core.
"""

import math

import ml_dtypes
import numpy as np

P = 128
NC = 8
D = 200
DP = 256          # padded feature dim (bf16 rows = 512B, dma_gather elem)
NSTEP = 6
NG = 256
GPC = NG // NC    # graphs per core
N_NODES = 50000
N_EDGES = 400000
ONES_ROW = 96     # partition index (block 1) of the ones (bias/mask) K row; must be 32-aligned
SLOT_BUDGET = 36  # max blocks (of [128, 256] bf16) per gather group tile
NSTRUCT = 4       # gather structures: source core pairs {2s, 2s+1}

BF16 = ml_dtypes.bfloat16


def _wrap_idx(arr):
    """Linear int16 index list -> [128, len/16] SBUF layout (16-part wrap,
    replicated 8x for the Q7 cores)."""
    assert arr.size % 16 == 0
    w = arr.reshape(-1, 16).T.astype(np.int16)  # [16, L/16]
    return np.tile(w, (8, 1))                   # [128, L/16]


def _preprocess(x, edge_index, batch, ndh=2):
    """Build per-core inputs + compile-time structure shared by all cores."""
    batch = np.asarray(batch).astype(np.int64)
    src = np.asarray(edge_index[0]).astype(np.int64)
    dst = np.asarray(edge_index[1]).astype(np.int64)
    n = batch.shape[0]

    node_core = batch // GPC
    gcounts = np.bincount(batch, minlength=NG).reshape(NC, GPC)
    gsize = gcounts.max(axis=0)                      # [GPC] cross-core aligned
    goff = 1 + np.concatenate([[0], np.cumsum(gsize)[:-1]])
    used = int(1 + gsize.sum())
    SH = ((used + 511) // 512) * 512
    assert 8 * SH <= 65536, f"table too big: 8*{SH}"
    NCH = SH // P

    # node -> local slot (graph-aligned), global table row
    first_of_graph = np.searchsorted(batch, np.arange(NG))
    rank = np.arange(n) - first_of_graph[batch]
    slot = goff[batch % GPC] + rank                  # [n], in [1, used)

    dst_core = node_core[dst]
    HSD = SH // ndh           # dst piece size (for pipelined aggregation)
    assert HSD % P == 0
    # 4 structures by SOURCE CORE PAIR: structure s covers source cores
    # {2s, 2s+1}. One merged AllGather produces T = [NC*SH, DP]; the view for
    # structure s is the contiguous slice T[2s*SH:(2s+2)*SH] (14336 rows,
    # int16-safe); view-local row = (src_core - 2s)*SH + src_slot. Row 0 of
    # every view (core 2s, slot 0) is a guaranteed-zero row.
    deg = np.zeros((NC, NSTRUCT, SH), np.int64)
    ell_rows = {}      # (c, s) -> (slot ids, view-local rows)
    src_slot = slot[src]
    src_core_a = node_core[src]
    for c in range(NC):
        m = dst_core == c
        ss = src_slot[m]
        sc = src_core_a[m]
        sd = slot[dst[m]]
        sid = sc // 2
        for s in range(NSTRUCT):
            ms = sid == s
            rows_s = (sc[ms] - 2 * s) * SH + ss[ms]
            slots_s = sd[ms]
            deg[c, s] = np.bincount(slots_s, minlength=SH)
            ell_rows[(c, s)] = (slots_s, rows_s)

    pad_row = [0] * NSTRUCT

    # Per-core degree-descending permutation per (structure, dst column piece).
    # Sorting within each dst piece keeps each piece's scatter self-contained
    # so piece d's aggregation/GRU can overlap piece d+1's gathers.
    perms = np.zeros((NC, NSTRUCT, SH), np.int64)
    deg_sorted = np.zeros((NC, NSTRUCT, SH), np.int64)
    for c in range(NC):
        for s in range(NSTRUCT):
            for d in range(ndh):
                dd = deg[c, s, d * HSD:(d + 1) * HSD]
                p_ = np.argsort(-dd, kind="stable") + d * HSD
                perms[c, s, d * HSD:(d + 1) * HSD] = p_
                deg_sorted[c, s, d * HSD:(d + 1) * HSD] = deg[c, s][p_]

    Kc = deg_sorted.max(axis=0)                      # [NSTRUCT, SH]
    NCHH = HSD // P                                  # chunks per dst piece
    Kch = Kc.reshape(NSTRUCT, ndh, NCHH, P).max(axis=3)  # [NSTRUCT, ndh, NCHH]

    # Group packing: consecutive chunks, G * maxK <= SLOT_BUDGET. DP minimizes
    # total gathered rows (padding) plus a fixed per-group overhead.
    OVR = 3000  # rows-equivalent fixed cost per gather group (Pool desc-gen
                # + fold/scatter chain serialization favors fewer groups)
    def _pack(caps):
        nz = np.nonzero(caps > 0)[0]
        if nz.size == 0:
            return []
        n = int(nz[-1]) + 1
        best = [0.0] + [float("inf")] * n
        choice = [0] * (n + 1)
        for j in range(1, n + 1):
            kmax = 0
            for i in range(j - 1, -1, -1):
                kmax = max(kmax, int(caps[i]))
                if (j - i) * max(kmax, 1) > SLOT_BUDGET:
                    break
                cost = best[i] + (j - i) * max(kmax, 1) * P + OVR
                if cost < best[j]:
                    best[j] = cost
                    choice[j] = i
        out = []
        j = n
        while j > 0:
            i = choice[j]
            out.append((i, j - i, max(int(caps[i:j].max()), 1)))
            j = i
        return out[::-1]

    groups = []                                      # (s, d, ch0, G, K)
    for d in range(ndh):
        for s in range(NSTRUCT):
            for (ch0, g, k) in _pack(Kch[s, d]):
                groups.append((s, d, ch0, g, k))

    # Build per-core ELL matrices [SH, Kmax] of view-local rows (0 = zero row).
    kmax_all = int(Kc.max())
    gidx_parts = {c: [] for c in range(NC)}
    sidx_parts = {c: [] for c in range(NC)}
    goffsets = []       # per group: (gidx col offset, sidx col offset)
    gcol = scol = 0
    ell = np.zeros((NC, NSTRUCT, SH, kmax_all), np.int64)
    for s in range(NSTRUCT):
        ell[:, s, :, :] = pad_row[s]
    for c in range(NC):
        for s in range(NSTRUCT):
            slots_s, rows_s = ell_rows[(c, s)]
            pos_of_slot = np.empty(SH, np.int64)
            pos_of_slot[perms[c, s]] = np.arange(SH)
            pos_e = pos_of_slot[slots_s]
            order = np.argsort(pos_e, kind="stable")
            pe, re_ = pos_e[order], rows_s[order]
            # within-position rank
            starts = np.searchsorted(pe, np.arange(SH))
            kk = np.arange(pe.size) - starts[pe]
            ell[c, s, pe, kk] = re_

    for (s, d, ch0, g, k) in groups:
        goffsets.append((gcol, scol))
        gcol += (g * k * P) // 16
        scol += (g * P) // 16
        p0 = d * HSD + ch0 * P
        for c in range(NC):
            sl = ell[c, s, p0:p0 + g * P, :k]              # [g*128, k]
            gidx_parts[c].append(_wrap_idx(sl.T.reshape(-1).astype(np.int16)))
            # scatter positions are local to the dst piece's aggd tile
            sp = (perms[c, s][p0:p0 + g * P] - d * HSD).astype(np.int16)
            sidx_parts[c].append(_wrap_idx(sp))

    per_core = []
    for c in range(NC):
        gi = np.concatenate(gidx_parts[c], axis=1) if gidx_parts[c] else np.zeros((128, 16), np.int16)
        si = np.concatenate(sidx_parts[c], axis=1)
        # initial hT (feature-major, bf16) and column mask
        hT0 = np.zeros((P, 2, SH), np.float32)
        mask = np.zeros((1, 2, SH), np.float32)
        nodes_c = np.nonzero(node_core == np.int64(c))[0]
        sl_c = slot[nodes_c]
        xc = np.asarray(x)[nodes_c]                  # [n_c, 200]
        hT0[:, 0, sl_c] = xc[:, 0:128].T
        hT0[0:D - 128, 1, sl_c] = xc[:, 128:D].T
        mask[0, :, sl_c] = 1.0
        hT0[ONES_ROW, 1, :] = mask[0, 0, :]
        per_core.append(dict(
            hT0=hT0.astype(BF16),
            gidx=gi.astype(np.int16),
            sidx=si.astype(np.int16),
        ))

    meta = dict(SH=SH, HS=SH // 2, HSD=HSD, NDH=ndh, NCH=NCH,
                groups=groups, goffsets=goffsets,
                gsize=gsize, goff=goff,
                gidx_cols=gcol, sidx_cols=scol)
    return per_core, meta


def _prep_weights(ggnn_weight, w_ih, w_hh, b_ih, b_hh, cls_w, cls_b):
    wt = np.zeros((NSTEP, 2, P, DP), np.float32)
    for t in range(NSTEP):
        wt[t, 0, :, 0:D] = ggnn_weight[t][0:128, :]
        wt[t, 1, 0:D - 128, 0:D] = ggnn_weight[t][128:D, :]

    def packT(w, b):
        o = np.zeros((2, P, 3 * D), np.float32)
        o[0, :, :] = w[:, 0:128].T
        o[1, 0:D - 128, :] = w[:, 128:D].T
        o[1, ONES_ROW, :] = b
        return o.astype(BF16)

    wih = packT(np.asarray(w_ih), np.asarray(b_ih))
    whh = packT(np.asarray(w_hh), np.asarray(b_hh))
    wcls = np.zeros((2, P, 2), np.float32)
    wcls[0] = np.asarray(cls_w)[:, 0:128].T
    wcls[1, 0:D - 128] = np.asarray(cls_w)[:, 128:D].T
    return dict(
        wt=wt.astype(BF16), wih=wih, whh=whh,
        wcls=wcls.astype(BF16),
        bcls=np.asarray(cls_b).reshape(2, 1).astype(np.float32),
    )


def _build_program(meta, parts=frozenset(('lib','cc','gs','sc','sh'))):
    import concourse.bacc as bacc
    import concourse.bass as bass
    import concourse.mybir as mybir
    import concourse.tile as tile
    from concourse.library_config import mlp

    SH, NCH = meta["SH"], meta["NCH"]
    HSD, NDH = meta["HSD"], meta["NDH"]
    groups, goffsets = meta["groups"], meta["goffsets"]
    gsize, goff = meta["gsize"], meta["goff"]
    # GRU column-chunk width: divide each dst piece into ~512-wide chunks
    NNC_D = max(1, round(HSD / 512))
    assert HSD % NNC_D == 0
    CW = HSD // NNC_D
    bf16, f32, i16 = mybir.dt.bfloat16, mybir.dt.float32, mybir.dt.int16
    AF = mybir.ActivationFunctionType

    nc = bacc.Bacc("TRN2")
    # --- I/O ---
    hT0_in = nc.dram_tensor("hT0", [P, 2, SH], bf16, kind="ExternalInput")
    gidx_in = nc.dram_tensor("gidx", [P, meta["gidx_cols"]], i16, kind="ExternalInput")
    sidx_in = nc.dram_tensor("sidx", [P, meta["sidx_cols"]], i16, kind="ExternalInput")
    aggz_in = nc.dram_tensor("aggz", [HSD + 16, DP], bf16, kind="ExternalInput")
    wt_in = nc.dram_tensor("wt", [NSTEP, 2, P, DP], bf16, kind="ExternalInput")
    wih_in = nc.dram_tensor("wih", [2, P, 3 * D], bf16, kind="ExternalInput")
    whh_in = nc.dram_tensor("whh", [2, P, 3 * D], bf16, kind="ExternalInput")
    wcls_in = nc.dram_tensor("wcls", [2, P, 2], bf16, kind="ExternalInput")
    bcls_in = nc.dram_tensor("bcls", [2, 1], f32, kind="ExternalInput")
    out_d = nc.dram_tensor("out", [2, GPC], f32, kind="ExternalOutput")

    from contextlib import ExitStack
    with tile.TileContext(nc) as tc, ExitStack() as ctx:
        const = ctx.enter_context(tc.tile_pool(name="const", bufs=1))
        dram = ctx.enter_context(tc.tile_pool(name="dram", bufs=3, space="DRAM"))
        hpool = ctx.enter_context(tc.tile_pool(name="hpool", bufs=1))
        slotp = ctx.enter_context(tc.tile_pool(name="slotp", bufs=3))
        mpool = ctx.enter_context(tc.tile_pool(name="mpool", bufs=1))
        gpool = ctx.enter_context(tc.tile_pool(name="gpool", bufs=3))
        psum_rz = ctx.enter_context(tc.tile_pool(name="psum_rz", bufs=2, space="PSUM"))
        psum_hn = ctx.enter_context(tc.tile_pool(name="psum_hn", bufs=2, space="PSUM"))

        if 'lib' in parts:
            nc.gpsimd.load_library(mlp)

        # --- load constants ---
        wt_t = const.tile([P, NSTEP * 2, DP], bf16)
        nc.sync.dma_start(wt_t[:], wt_in.rearrange("t b p e -> p (t b) e"))
        wih_t = const.tile([P, 2, 3 * D], bf16)
        nc.sync.dma_start(wih_t[:], wih_in.rearrange("b p m -> p b m"))
        whh_t = const.tile([P, 2, 3 * D], bf16)
        nc.sync.dma_start(whh_t[:], whh_in.rearrange("b p m -> p b m"))
        wcls_t = const.tile([P, 2, 2], bf16)
        nc.sync.dma_start(wcls_t[:], wcls_in.rearrange("b p m -> p b m"))
        bcls_t = const.tile([2, 1], f32)
        nc.sync.dma_start(bcls_t[:], bcls_in[:])
        gidx_t = const.tile([P, meta["gidx_cols"]], i16)
        nc.sync.dma_start(gidx_t[:], gidx_in[:])
        sidx_t = const.tile([P, meta["sidx_cols"]], i16)
        nc.sync.dma_start(sidx_t[:], sidx_in[:])

        hA = hpool.tile([P, 2, SH], bf16, name="hA")
        hB = hpool.tile([P, 2, SH], bf16, name="hB")
        nc.sync.dma_start(hA[:], hT0_in[:])
        nc.vector.memset(hB[64:P, 1, :], 0.0)
        nc.vector.tensor_copy(hB[ONES_ROW:ONES_ROW + 1, 1, :],
                              hA[ONES_ROW:ONES_ROW + 1, 1, :])

        K1 = D - 128  # 72

        import os as _os
        REP = int(_os.environ.get("KREPEAT", "1"))
        for t in range(NSTEP * REP):
            h_old = hA if t % 2 == 0 else hB
            h_new = hB if t % 2 == 0 else hA

            mbb = dram.tile([SH, DP], bf16, name="mbb", tag="mbb")
            tab = dram.tile([NC * SH, DP], bf16, name="tab", tag="tab",
                            addr_space="Shared" if 'sh' in parts else "Local")
            aggds = [dram.tile([HSD + 16, DP], bf16, name=f"aggd{d}", tag=f"aggd{d}")
                     for d in range(NDH)]
            for d in range(NDH):
                nc.sync.dma_start(aggds[d][:], aggz_in[:])

            # --- m = h @ W_t  (node-major, bf16, padded cols), staged in
            # 2*NDH pieces so each mbb slice lands right after its GRU_{t-1}
            # columns are done (shortens the GRU->AllGather handoff) ---
            NMP = 2 * NDH
            MCH = NCH // NMP
            MR = SH // NMP
            mts = [mpool.tile([P, MCH, DP], bf16, name=f"mt{half}", tag=f"mt{half}")
                   for half in range(NMP)]
            for half in range(NMP):
                mt = mts[half]
                for ch in range(MCH):
                    mp = psum_hn.tile([P, DP], f32, name="mp", tag="hn")
                    cols = slice((half * MCH + ch) * P, (half * MCH + ch + 1) * P)
                    nc.tensor.matmul(out=mp[:], lhsT=h_old[:, 0, cols],
                                     rhs=wt_t[:, (t % NSTEP) * 2, :], start=True, stop=False)
                    nc.tensor.matmul(out=mp[:], lhsT=h_old[0:K1, 1, cols],
                                     rhs=wt_t[0:K1, (t % NSTEP) * 2 + 1, :], start=False, stop=True)
                    nc.any.tensor_copy(mt[:, ch, :], mp[:])
                nc.sync.dma_start(
                    mbb[half * MR:(half + 1) * MR, :].rearrange("(c p) e -> p c e", p=P),
                    mt[:])

            # --- AllGather m (one merged collective: bigger transfers run at
            # much higher effective bandwidth than two half-table ones) ---
            if 'cc' in parts:
                nc.gpsimd.collective_compute(
                    "AllGather", mybir.AluOpType.bypass,
                    replica_groups=[list(range(NC))],
                    ins=[mbb.opt()], outs=[tab.opt()],
                )

            # --- per dst piece: gather + fold + scatter-add, transpose, GRU.
            # Piece d+1's gathers/folds overlap piece d's transpose + GRU. ---
            aggT = mpool.tile([P, 2, SH], bf16, name="aggT", tag="aggT")
            import os as _os
            _ng = int(_os.environ.get("KGROUPS", "999"))
            # All gather groups first (d-ordered): piece d+1's gathers/folds
            # queue ahead of piece d's GRU work on DVE, so they overlap it.
            for gi_, (s, gd, ch0, g, k) in enumerate((groups if 'gs' in parts else [])[:_ng]):
                gcol, scol = goffsets[gi_]
                nidx = g * k * P
                tg = slotp.tile([P, SLOT_BUDGET, DP], bf16, name="tg", tag="tg")
                view = tab[2 * s * SH:(2 * s + 2) * SH, :]
                nc.gpsimd.dma_gather(
                    tg[:, 0:g * k, :], view,
                    gidx_t[:, gcol:gcol + nidx // 16],
                    nidx, nidx, DP, single_packet=False,
                )
                kk = k
                while kk > 1:
                    hi = (kk + 1) // 2
                    lo = kk // 2
                    nc.vector.tensor_add(
                        out=tg[:, 0:lo * g, 0:D],
                        in0=tg[:, 0:lo * g, 0:D],
                        in1=tg[:, hi * g:(hi + lo) * g, 0:D],
                    )
                    kk = hi
                if 'sc' in parts:
                    nc.gpsimd.dma_scatter_add(
                        aggds[gd][0:HSD, :], tg[:, 0:g, :],
                        sidx_t[:, scol:scol + (g * P) // 16],
                        g * P, g * P, DP, single_packet=False,
                    )

            for d in range(NDH):
                aggd = aggds[d]
                # --- transpose this piece's agg to feature-major ---
                dsl = slice(d * HSD, (d + 1) * HSD)
                nc.sync.dma_start(out=aggT[:, 0, dsl], in_=aggd[0:HSD, 0:128],
                                  transpose=True)
                nc.sync.dma_start(out=aggT[:, 1, dsl], in_=aggd[0:HSD, 128:256],
                                  transpose=True)
                nc.vector.tensor_copy(aggT[ONES_ROW:ONES_ROW + 1, 1, dsl],
                                      h_old[ONES_ROW:ONES_ROW + 1, 1, dsl])

                # --- GRU for this piece's columns ---
                for i in range(NNC_D):
                    cols = slice(d * HSD + i * CW, d * HSD + (i + 1) * CW)
                    for mb in range(2):
                        mr = P if mb == 0 else K1
                        gsl = [slice(gg * D + mb * 128, gg * D + mb * 128 + mr) for gg in range(3)]
                        rz = psum_rz.tile([P, 2 * CW], f32, name="rz", tag="rz")
                        for half, gg in ((0, 0), (1, 1)):  # r, z gates
                            o = rz[0:mr, half * CW:(half + 1) * CW]
                            nc.tensor.matmul(out=o, lhsT=wih_t[:, 0, gsl[gg]],
                                             rhs=aggT[:, 0, cols], start=True, stop=False)
                            nc.tensor.matmul(out=o, lhsT=wih_t[0:ONES_ROW + 1, 1, gsl[gg]],
                                             rhs=aggT[0:ONES_ROW + 1, 1, cols], start=False, stop=False)
                            nc.tensor.matmul(out=o, lhsT=whh_t[:, 0, gsl[gg]],
                                             rhs=h_old[:, 0, cols], start=False, stop=False)
                            nc.tensor.matmul(out=o, lhsT=whh_t[0:ONES_ROW + 1, 1, gsl[gg]],
                                             rhs=h_old[0:ONES_ROW + 1, 1, cols], start=False, stop=True)
                        rzs = gpool.tile([P, 2 * CW], bf16, name="rzs", tag="rzs")
                        nc.scalar.activation(rzs[0:mr, :], rz[0:mr, :], AF.Sigmoid)

                        hn = psum_hn.tile([P, 2 * CW], f32, name="hn", tag="hn")
                        nc.tensor.matmul(out=hn[0:mr, 0:CW], lhsT=whh_t[:, 0, gsl[2]],
                                         rhs=h_old[:, 0, cols], start=True, stop=False)
                        nc.tensor.matmul(out=hn[0:mr, 0:CW], lhsT=whh_t[0:ONES_ROW + 1, 1, gsl[2]],
                                         rhs=h_old[0:ONES_ROW + 1, 1, cols], start=False, stop=True)
                        nc.tensor.matmul(out=hn[0:mr, CW:2 * CW], lhsT=wih_t[:, 0, gsl[2]],
                                         rhs=aggT[:, 0, cols], start=True, stop=False)
                        nc.tensor.matmul(out=hn[0:mr, CW:2 * CW], lhsT=wih_t[0:ONES_ROW + 1, 1, gsl[2]],
                                         rhs=aggT[0:ONES_ROW + 1, 1, cols], start=False, stop=True)

                        hns = gpool.tile([P, CW], bf16, name="hns", tag="hns")
                        nc.any.tensor_copy(hns[0:mr, :], hn[0:mr, 0:CW])
                        rhn = gpool.tile([P, CW], bf16, name="rhn", tag="rhn")
                        nc.vector.tensor_mul(rhn[0:mr, :], rzs[0:mr, 0:CW], hns[0:mr, :])
                        nc.vector.tensor_add(hn[0:mr, CW:2 * CW], hn[0:mr, CW:2 * CW],
                                             rhn[0:mr, :])
                        nt = gpool.tile([P, CW], bf16, name="nt", tag="nt")
                        nc.scalar.activation(nt[0:mr, :], hn[0:mr, CW:2 * CW], AF.Tanh)

                        t1 = gpool.tile([P, CW], bf16, name="t1", tag="t1")
                        nc.vector.tensor_sub(t1[0:mr, :], h_old[0:mr, mb, cols], nt[0:mr, :])
                        nc.vector.tensor_mul(t1[0:mr, :], rzs[0:mr, CW:2 * CW], t1[0:mr, :])
                        nc.vector.tensor_add(h_new[0:mr, mb, cols], nt[0:mr, :], t1[0:mr, :])


        # --- pooling + classifier ---
        h_fin = hA if (NSTEP * REP) % 2 == 0 else hB
        pooled = gpool.tile([P, 2, GPC], bf16, name="pooled", bufs=1)
        for g in range(GPC):
            a, b = int(goff[g]), int(goff[g] + gsize[g])
            nc.vector.tensor_reduce(pooled[:, :, g:g + 1], h_fin[:, :, a:b],
                                    axis=mybir.AxisListType.X,
                                    op=mybir.AluOpType.max)
        nc.vector.tensor_scalar_max(pooled[:], pooled[:], 0.0)
        lg = psum_hn.tile([2, GPC], f32, name="lg", tag="hn")
        nc.tensor.matmul(out=lg[:], lhsT=wcls_t[:, 0, :], rhs=pooled[:, 0, :],
                         start=True, stop=False)
        nc.tensor.matmul(out=lg[:], lhsT=wcls_t[0:K1, 1, :], rhs=pooled[0:K1, 1, :],
                         start=False, stop=True)
        ot = gpool.tile([2, GPC], f32, name="ot", bufs=1)
        nc.scalar.activation(ot[:], lg[:], AF.Sigmoid, bias=bcls_t[:])
        nc.sync.dma_start(out_d[:], ot[:])

    nc.compile()
    return nc


def kernel(**inputs):
    import os
    from concourse.bass_utils import run_bass_kernel_spmd

    per_core, meta = _preprocess(inputs["x"], inputs["edge_index"], inputs["batch"])
    w = _prep_weights(inputs["ggnn_weight"], inputs["w_ih"], inputs["w_hh"],
                      inputs["b_ih"], inputs["b_hh"], inputs["cls_w"], inputs["cls_b"])
    aggz = np.zeros((meta["HSD"] + 16, DP), BF16)
    in_maps = [dict(hT0=pc["hT0"], gidx=pc["gidx"],
                    sidx=pc["sidx"], aggz=aggz, **w) for pc in per_core]
    nc = _build_program(meta)
    trace = bool(int(os.environ.get("KTRACE", "0")))
    res = run_bass_kernel_spmd(nc, in_maps, core_ids=list(range(NC)), trace=trace)
    if trace:
        print(f"HW exec time: {res.exec_time_ns} ns")
        print("trace:", res.instructions_and_trace[1] if res.instructions_and_trace else None)
    out = np.zeros((NG, 2), np.float32)
    for c in range(NC):
        out[c * GPC:(c + 1) * GPC, :] = res.results[c]["out"].T
    return out



# revision 24
# speedup vs baseline: 2.5067x; 2.5067x over previous
"""GGNN (Devign) message-passing kernel for 8 Trainium2 NeuronCores.

Strategy (graph-parallel, dst-sharded):
  - Nodes are sharded across 8 cores at graph boundaries (32 graphs/core) so
    max-pooling stays local. Node columns are graph-aligned (cross-core common
    per-graph stride) so pooling ranges are compile-time constants (SPMD).
  - Per step: each core computes m = h @ W for its shard (node-major, padded
    256-col bf16 rows), AllGathers m into a shared 8*SH-row table in DRAM,
    gathers the rows for its in-edges with bulk dma_gather (two int16-indexed
    views of the table, split by source row < / >= 4*SH), segment-sums them
    with log-depth DVE folds over a degree-sorted ELL layout, scatter-adds
    per-node partials into a local agg table (dma_scatter_add), DMA-transposes
    agg into feature-major SBUF, and runs the GRU with bf16 matmuls (f32 PSUM
    accum, biases folded in via a constant ones-row at K index 72 of block 1).
  - After 6 steps: mask junk columns, per-graph reduce_max pooling, ReLU,
    2x200 classifier matmul, sigmoid, output [2, 32] per core.
"""

import math

import ml_dtypes
import numpy as np

P = 128
NC = 8
D = 200
DP = 256          # padded feature dim (bf16 rows = 512B, dma_gather elem)
NSTEP = 6
NG = 256
GPC = NG // NC    # graphs per core
N_NODES = 50000
N_EDGES = 400000
ONES_ROW = 96     # partition index (block 1) of the ones (bias/mask) K row; must be 32-aligned
SLOT_BUDGET = 36  # max blocks (of [128, 256] bf16) per gather group tile
NSTRUCT = 4       # gather structures: source core pairs {2s, 2s+1}

BF16 = ml_dtypes.bfloat16


def _wrap_idx(arr):
    """Linear int16 index list -> [128, len/16] SBUF layout (16-part wrap,
    replicated 8x for the Q7 cores)."""
    assert arr.size % 16 == 0
    w = arr.reshape(-1, 16).T.astype(np.int16)  # [16, L/16]
    return np.tile(w, (8, 1))                   # [128, L/16]


def _preprocess(x, edge_index, batch, ndh=2):
    """Build per-core inputs + compile-time structure shared by all cores."""
    batch = np.asarray(batch).astype(np.int64)
    src = np.asarray(edge_index[0]).astype(np.int64)
    dst = np.asarray(edge_index[1]).astype(np.int64)
    n = batch.shape[0]

    node_core = batch // GPC
    gcounts = np.bincount(batch, minlength=NG).reshape(NC, GPC)
    gsize = gcounts.max(axis=0)                      # [GPC] cross-core aligned
    goff = 1 + np.concatenate([[0], np.cumsum(gsize)[:-1]])
    used = int(1 + gsize.sum())
    SH = ((used + 511) // 512) * 512
    assert 8 * SH <= 65536, f"table too big: 8*{SH}"
    NCH = SH // P

    # node -> local slot (graph-aligned), global table row
    first_of_graph = np.searchsorted(batch, np.arange(NG))
    rank = np.arange(n) - first_of_graph[batch]
    slot = goff[batch % GPC] + rank                  # [n], in [1, used)

    dst_core = node_core[dst]
    HSD = SH // ndh           # dst piece size (for pipelined aggregation)
    assert HSD % P == 0
    # 4 structures by SOURCE CORE PAIR: structure s covers source cores
    # {2s, 2s+1}. One merged AllGather produces T = [NC*SH, DP]; the view for
    # structure s is the contiguous slice T[2s*SH:(2s+2)*SH] (14336 rows,
    # int16-safe); view-local row = (src_core - 2s)*SH + src_slot. Row 0 of
    # every view (core 2s, slot 0) is a guaranteed-zero row.
    deg = np.zeros((NC, NSTRUCT, SH), np.int64)
    ell_rows = {}      # (c, s) -> (slot ids, view-local rows)
    src_slot = slot[src]
    src_core_a = node_core[src]
    for c in range(NC):
        m = dst_core == c
        ss = src_slot[m]
        sc = src_core_a[m]
        sd = slot[dst[m]]
        sid = sc // 2
        for s in range(NSTRUCT):
            ms = sid == s
            rows_s = (sc[ms] - 2 * s) * SH + ss[ms]
            slots_s = sd[ms]
            deg[c, s] = np.bincount(slots_s, minlength=SH)
            ell_rows[(c, s)] = (slots_s, rows_s)

    pad_row = [0] * NSTRUCT

    # Per-core degree-descending permutation per (structure, dst column piece).
    # Sorting within each dst piece keeps each piece's scatter self-contained
    # so piece d's aggregation/GRU can overlap piece d+1's gathers.
    perms = np.zeros((NC, NSTRUCT, SH), np.int64)
    deg_sorted = np.zeros((NC, NSTRUCT, SH), np.int64)
    for c in range(NC):
        for s in range(NSTRUCT):
            for d in range(ndh):
                dd = deg[c, s, d * HSD:(d + 1) * HSD]
                p_ = np.argsort(-dd, kind="stable") + d * HSD
                perms[c, s, d * HSD:(d + 1) * HSD] = p_
                deg_sorted[c, s, d * HSD:(d + 1) * HSD] = deg[c, s][p_]

    Kc = deg_sorted.max(axis=0)                      # [NSTRUCT, SH]
    NCHH = HSD // P                                  # chunks per dst piece
    Kch = Kc.reshape(NSTRUCT, ndh, NCHH, P).max(axis=3)  # [NSTRUCT, ndh, NCHH]

    # Group packing: consecutive chunks, G * maxK <= SLOT_BUDGET. DP minimizes
    # total gathered rows (padding) plus a fixed per-group overhead.
    OVR = 3000  # rows-equivalent fixed cost per gather group (Pool desc-gen
                # + fold/scatter chain serialization favors fewer groups)
    def _pack(caps):
        nz = np.nonzero(caps > 0)[0]
        if nz.size == 0:
            return []
        n = int(nz[-1]) + 1
        best = [0.0] + [float("inf")] * n
        choice = [0] * (n + 1)
        for j in range(1, n + 1):
            kmax = 0
            for i in range(j - 1, -1, -1):
                kmax = max(kmax, int(caps[i]))
                if (j - i) * max(kmax, 1) > SLOT_BUDGET:
                    break
                cost = best[i] + (j - i) * max(kmax, 1) * P + OVR
                if cost < best[j]:
                    best[j] = cost
                    choice[j] = i
        out = []
        j = n
        while j > 0:
            i = choice[j]
            out.append((i, j - i, max(int(caps[i:j].max()), 1)))
            j = i
        return out[::-1]

    groups = []                                      # (s, d, ch0, G, K)
    for d in range(ndh):
        for s in range(NSTRUCT):
            for (ch0, g, k) in _pack(Kch[s, d]):
                groups.append((s, d, ch0, g, k))

    # Build per-core ELL matrices [SH, Kmax] of view-local rows (0 = zero row).
    kmax_all = int(Kc.max())
    gidx_parts = {c: [] for c in range(NC)}
    sidx_parts = {c: [] for c in range(NC)}
    goffsets = []       # per group: (gidx col offset, sidx col offset)
    gcol = scol = 0
    ell = np.zeros((NC, NSTRUCT, SH, kmax_all), np.int64)
    for s in range(NSTRUCT):
        ell[:, s, :, :] = pad_row[s]
    for c in range(NC):
        for s in range(NSTRUCT):
            slots_s, rows_s = ell_rows[(c, s)]
            pos_of_slot = np.empty(SH, np.int64)
            pos_of_slot[perms[c, s]] = np.arange(SH)
            pos_e = pos_of_slot[slots_s]
            order = np.argsort(pos_e, kind="stable")
            pe, re_ = pos_e[order], rows_s[order]
            # within-position rank
            starts = np.searchsorted(pe, np.arange(SH))
            kk = np.arange(pe.size) - starts[pe]
            ell[c, s, pe, kk] = re_

    for (s, d, ch0, g, k) in groups:
        goffsets.append((gcol, scol))
        gcol += (g * k * P) // 16
        scol += (g * P) // 16
        p0 = d * HSD + ch0 * P
        for c in range(NC):
            sl = ell[c, s, p0:p0 + g * P, :k]              # [g*128, k]
            gidx_parts[c].append(_wrap_idx(sl.T.reshape(-1).astype(np.int16)))
            # scatter positions are local to the dst piece's aggd tile
            sp = (perms[c, s][p0:p0 + g * P] - d * HSD).astype(np.int16)
            sidx_parts[c].append(_wrap_idx(sp))

    per_core = []
    for c in range(NC):
        gi = np.concatenate(gidx_parts[c], axis=1) if gidx_parts[c] else np.zeros((128, 16), np.int16)
        si = np.concatenate(sidx_parts[c], axis=1)
        # initial hT (feature-major, bf16) and column mask
        hT0 = np.zeros((P, 2, SH), np.float32)
        mask = np.zeros((1, 2, SH), np.float32)
        nodes_c = np.nonzero(node_core == np.int64(c))[0]
        sl_c = slot[nodes_c]
        xc = np.asarray(x)[nodes_c]                  # [n_c, 200]
        hT0[:, 0, sl_c] = xc[:, 0:128].T
        hT0[0:D - 128, 1, sl_c] = xc[:, 128:D].T
        mask[0, :, sl_c] = 1.0
        hT0[ONES_ROW, 1, :] = mask[0, 0, :]
        per_core.append(dict(
            hT0=hT0.astype(BF16),
            gidx=gi.astype(np.int16),
            sidx=si.astype(np.int16),
        ))

    meta = dict(SH=SH, HS=SH // 2, HSD=HSD, NDH=ndh, NCH=NCH,
                groups=groups, goffsets=goffsets,
                gsize=gsize, goff=goff,
                gidx_cols=gcol, sidx_cols=scol)
    return per_core, meta


def _prep_weights(ggnn_weight, w_ih, w_hh, b_ih, b_hh, cls_w, cls_b):
    wt = np.zeros((NSTEP, 2, P, DP), np.float32)
    for t in range(NSTEP):
        wt[t, 0, :, 0:D] = ggnn_weight[t][0:128, :]
        wt[t, 1, 0:D - 128, 0:D] = ggnn_weight[t][128:D, :]

    def packT(w, b):
        o = np.zeros((2, P, 3 * D), np.float32)
        o[0, :, :] = w[:, 0:128].T
        o[1, 0:D - 128, :] = w[:, 128:D].T
        o[1, ONES_ROW, :] = b
        return o.astype(BF16)

    wih = packT(np.asarray(w_ih), np.asarray(b_ih))
    whh = packT(np.asarray(w_hh), np.asarray(b_hh))
    wcls = np.zeros((2, P, 2), np.float32)
    wcls[0] = np.asarray(cls_w)[:, 0:128].T
    wcls[1, 0:D - 128] = np.asarray(cls_w)[:, 128:D].T
    return dict(
        wt=wt.astype(BF16), wih=wih, whh=whh,
        wcls=wcls.astype(BF16),
        bcls=np.asarray(cls_b).reshape(2, 1).astype(np.float32),
    )


def _build_program(meta, parts=frozenset(('lib','cc','gs','sc','sh'))):
    import concourse.bacc as bacc
    import concourse.bass as bass
    import concourse.mybir as mybir
    import concourse.tile as tile
    from concourse.library_config import mlp

    SH, NCH = meta["SH"], meta["NCH"]
    HSD, NDH = meta["HSD"], meta["NDH"]
    groups, goffsets = meta["groups"], meta["goffsets"]
    gsize, goff = meta["gsize"], meta["goff"]
    # GRU column-chunk width: divide each dst piece into ~512-wide chunks
    NNC_D = max(1, round(HSD / 512))
    assert HSD % NNC_D == 0
    CW = HSD // NNC_D
    bf16, f32, i16 = mybir.dt.bfloat16, mybir.dt.float32, mybir.dt.int16
    AF = mybir.ActivationFunctionType

    # 4 SWDGE queues: gathers round-robin queues 0-2 (independent reads,
    # 3x the per-queue DMA service rate), scatter-adds pinned to queue 3 so
    # RMW updates to the same aggd rows stay serialized among themselves.
    nc = bacc.Bacc("TRN2", num_swdge_queues=4)
    # --- I/O ---
    hT0_in = nc.dram_tensor("hT0", [P, 2, SH], bf16, kind="ExternalInput")
    gidx_in = nc.dram_tensor("gidx", [P, meta["gidx_cols"]], i16, kind="ExternalInput")
    sidx_in = nc.dram_tensor("sidx", [P, meta["sidx_cols"]], i16, kind="ExternalInput")
    aggz_in = nc.dram_tensor("aggz", [HSD + 16, DP], bf16, kind="ExternalInput")
    wt_in = nc.dram_tensor("wt", [NSTEP, 2, P, DP], bf16, kind="ExternalInput")
    wih_in = nc.dram_tensor("wih", [2, P, 3 * D], bf16, kind="ExternalInput")
    whh_in = nc.dram_tensor("whh", [2, P, 3 * D], bf16, kind="ExternalInput")
    wcls_in = nc.dram_tensor("wcls", [2, P, 2], bf16, kind="ExternalInput")
    bcls_in = nc.dram_tensor("bcls", [2, 1], f32, kind="ExternalInput")
    out_d = nc.dram_tensor("out", [2, GPC], f32, kind="ExternalOutput")

    from contextlib import ExitStack
    with tile.TileContext(nc) as tc, ExitStack() as ctx:
        const = ctx.enter_context(tc.tile_pool(name="const", bufs=1))
        dram = ctx.enter_context(tc.tile_pool(name="dram", bufs=3, space="DRAM"))
        hpool = ctx.enter_context(tc.tile_pool(name="hpool", bufs=1))
        slotp = ctx.enter_context(tc.tile_pool(name="slotp", bufs=3))
        mpool = ctx.enter_context(tc.tile_pool(name="mpool", bufs=1))
        gpool = ctx.enter_context(tc.tile_pool(name="gpool", bufs=3))
        psum_rz = ctx.enter_context(tc.tile_pool(name="psum_rz", bufs=2, space="PSUM"))
        psum_hn = ctx.enter_context(tc.tile_pool(name="psum_hn", bufs=2, space="PSUM"))

        if 'lib' in parts:
            nc.gpsimd.load_library(mlp)

        # --- load constants ---
        wt_t = const.tile([P, NSTEP * 2, DP], bf16)
        nc.sync.dma_start(wt_t[:], wt_in.rearrange("t b p e -> p (t b) e"))
        wih_t = const.tile([P, 2, 3 * D], bf16)
        nc.sync.dma_start(wih_t[:], wih_in.rearrange("b p m -> p b m"))
        whh_t = const.tile([P, 2, 3 * D], bf16)
        nc.sync.dma_start(whh_t[:], whh_in.rearrange("b p m -> p b m"))
        wcls_t = const.tile([P, 2, 2], bf16)
        nc.sync.dma_start(wcls_t[:], wcls_in.rearrange("b p m -> p b m"))
        bcls_t = const.tile([2, 1], f32)
        nc.sync.dma_start(bcls_t[:], bcls_in[:])
        gidx_t = const.tile([P, meta["gidx_cols"]], i16)
        nc.sync.dma_start(gidx_t[:], gidx_in[:])
        sidx_t = const.tile([P, meta["sidx_cols"]], i16)
        nc.sync.dma_start(sidx_t[:], sidx_in[:])

        hA = hpool.tile([P, 2, SH], bf16, name="hA")
        hB = hpool.tile([P, 2, SH], bf16, name="hB")
        nc.sync.dma_start(hA[:], hT0_in[:])
        nc.vector.memset(hB[64:P, 1, :], 0.0)
        nc.vector.tensor_copy(hB[ONES_ROW:ONES_ROW + 1, 1, :],
                              hA[ONES_ROW:ONES_ROW + 1, 1, :])

        K1 = D - 128  # 72

        import os as _os
        REP = int(_os.environ.get("KREPEAT", "1"))
        for t in range(NSTEP * REP):
            h_old = hA if t % 2 == 0 else hB
            h_new = hB if t % 2 == 0 else hA

            mbb = dram.tile([SH, DP], bf16, name="mbb", tag="mbb")
            tab = dram.tile([NC * SH, DP], bf16, name="tab", tag="tab",
                            addr_space="Shared" if 'sh' in parts else "Local")
            aggds = [dram.tile([HSD + 16, DP], bf16, name=f"aggd{d}", tag=f"aggd{d}")
                     for d in range(NDH)]
            for d in range(NDH):
                nc.sync.dma_start(aggds[d][:], aggz_in[:])

            # --- m = h @ W_t  (node-major, bf16, padded cols), staged in
            # 2*NDH pieces so each mbb slice lands right after its GRU_{t-1}
            # columns are done (shortens the GRU->AllGather handoff) ---
            NMP = 2 * NDH
            MCH = NCH // NMP
            MR = SH // NMP
            mts = [mpool.tile([P, MCH, DP], bf16, name=f"mt{half}", tag=f"mt{half}")
                   for half in range(NMP)]
            for half in range(NMP):
                mt = mts[half]
                for ch in range(MCH):
                    mp = psum_hn.tile([P, DP], f32, name="mp", tag="hn")
                    cols = slice((half * MCH + ch) * P, (half * MCH + ch + 1) * P)
                    nc.tensor.matmul(out=mp[:], lhsT=h_old[:, 0, cols],
                                     rhs=wt_t[:, (t % NSTEP) * 2, :], start=True, stop=False)
                    nc.tensor.matmul(out=mp[:], lhsT=h_old[0:K1, 1, cols],
                                     rhs=wt_t[0:K1, (t % NSTEP) * 2 + 1, :], start=False, stop=True)
                    nc.any.tensor_copy(mt[:, ch, :], mp[:])
                nc.sync.dma_start(
                    mbb[half * MR:(half + 1) * MR, :].rearrange("(c p) e -> p c e", p=P),
                    mt[:])

            # --- AllGather m (one merged collective: bigger transfers run at
            # much higher effective bandwidth than two half-table ones) ---
            if 'cc' in parts:
                nc.gpsimd.collective_compute(
                    "AllGather", mybir.AluOpType.bypass,
                    replica_groups=[list(range(NC))],
                    ins=[mbb.opt()], outs=[tab.opt()],
                )

            # --- per dst piece: gather + fold + scatter-add, transpose, GRU.
            # Piece d+1's gathers/folds overlap piece d's transpose + GRU. ---
            aggT = mpool.tile([P, 2, SH], bf16, name="aggT", tag="aggT")
            import os as _os
            _ng = int(_os.environ.get("KGROUPS", "999"))
            # All gather groups first (d-ordered): piece d+1's gathers/folds
            # queue ahead of piece d's GRU work on DVE, so they overlap it.
            for gi_, (s, gd, ch0, g, k) in enumerate((groups if 'gs' in parts else [])[:_ng]):
                gcol, scol = goffsets[gi_]
                nidx = g * k * P
                tg = slotp.tile([P, SLOT_BUDGET, DP], bf16, name="tg", tag="tg")
                view = tab[2 * s * SH:(2 * s + 2) * SH, :]
                nc.gpsimd.dma_gather(
                    tg[:, 0:g * k, :], view,
                    gidx_t[:, gcol:gcol + nidx // 16],
                    nidx, nidx, DP, single_packet=False,
                    queue_num=gi_ % 3,
                )
                kk = k
                while kk > 1:
                    hi = (kk + 1) // 2
                    lo = kk // 2
                    nc.vector.tensor_add(
                        out=tg[:, 0:lo * g, 0:D],
                        in0=tg[:, 0:lo * g, 0:D],
                        in1=tg[:, hi * g:(hi + lo) * g, 0:D],
                    )
                    kk = hi
                if 'sc' in parts:
                    nc.gpsimd.dma_scatter_add(
                        aggds[gd][0:HSD, :], tg[:, 0:g, :],
                        sidx_t[:, scol:scol + (g * P) // 16],
                        g * P, g * P, DP, single_packet=False,
                        queue_num=3,
                    )

            for d in range(NDH):
                aggd = aggds[d]
                # --- transpose this piece's agg to feature-major ---
                dsl = slice(d * HSD, (d + 1) * HSD)
                nc.sync.dma_start(out=aggT[:, 0, dsl], in_=aggd[0:HSD, 0:128],
                                  transpose=True)
                nc.sync.dma_start(out=aggT[:, 1, dsl], in_=aggd[0:HSD, 128:256],
                                  transpose=True)
                nc.vector.tensor_copy(aggT[ONES_ROW:ONES_ROW + 1, 1, dsl],
                                      h_old[ONES_ROW:ONES_ROW + 1, 1, dsl])

                # --- GRU for this piece's columns ---
                for i in range(NNC_D):
                    cols = slice(d * HSD + i * CW, d * HSD + (i + 1) * CW)
                    for mb in range(2):
                        mr = P if mb == 0 else K1
                        gsl = [slice(gg * D + mb * 128, gg * D + mb * 128 + mr) for gg in range(3)]
                        rz = psum_rz.tile([P, 2 * CW], f32, name="rz", tag="rz")
                        for half, gg in ((0, 0), (1, 1)):  # r, z gates
                            o = rz[0:mr, half * CW:(half + 1) * CW]
                            nc.tensor.matmul(out=o, lhsT=wih_t[:, 0, gsl[gg]],
                                             rhs=aggT[:, 0, cols], start=True, stop=False)
                            nc.tensor.matmul(out=o, lhsT=wih_t[0:ONES_ROW + 1, 1, gsl[gg]],
                                             rhs=aggT[0:ONES_ROW + 1, 1, cols], start=False, stop=False)
                            nc.tensor.matmul(out=o, lhsT=whh_t[:, 0, gsl[gg]],
                                             rhs=h_old[:, 0, cols], start=False, stop=False)
                            nc.tensor.matmul(out=o, lhsT=whh_t[0:ONES_ROW + 1, 1, gsl[gg]],
                                             rhs=h_old[0:ONES_ROW + 1, 1, cols], start=False, stop=True)
                        rzs = gpool.tile([P, 2 * CW], bf16, name="rzs", tag="rzs")
                        nc.scalar.activation(rzs[0:mr, :], rz[0:mr, :], AF.Sigmoid)

                        hn = psum_hn.tile([P, 2 * CW], f32, name="hn", tag="hn")
                        nc.tensor.matmul(out=hn[0:mr, 0:CW], lhsT=whh_t[:, 0, gsl[2]],
                                         rhs=h_old[:, 0, cols], start=True, stop=False)
                        nc.tensor.matmul(out=hn[0:mr, 0:CW], lhsT=whh_t[0:ONES_ROW + 1, 1, gsl[2]],
                                         rhs=h_old[0:ONES_ROW + 1, 1, cols], start=False, stop=True)
                        nc.tensor.matmul(out=hn[0:mr, CW:2 * CW], lhsT=wih_t[:, 0, gsl[2]],
                                         rhs=aggT[:, 0, cols], start=True, stop=False)
                        nc.tensor.matmul(out=hn[0:mr, CW:2 * CW], lhsT=wih_t[0:ONES_ROW + 1, 1, gsl[2]],
                                         rhs=aggT[0:ONES_ROW + 1, 1, cols], start=False, stop=True)

                        hns = gpool.tile([P, CW], bf16, name="hns", tag="hns")
                        nc.any.tensor_copy(hns[0:mr, :], hn[0:mr, 0:CW])
                        rhn = gpool.tile([P, CW], bf16, name="rhn", tag="rhn")
                        nc.vector.tensor_mul(rhn[0:mr, :], rzs[0:mr, 0:CW], hns[0:mr, :])
                        nc.vector.tensor_add(hn[0:mr, CW:2 * CW], hn[0:mr, CW:2 * CW],
                                             rhn[0:mr, :])
                        nt = gpool.tile([P, CW], bf16, name="nt", tag="nt")
                        nc.scalar.activation(nt[0:mr, :], hn[0:mr, CW:2 * CW], AF.Tanh)

                        t1 = gpool.tile([P, CW], bf16, name="t1", tag="t1")
                        nc.vector.tensor_sub(t1[0:mr, :], h_old[0:mr, mb, cols], nt[0:mr, :])
                        nc.vector.tensor_mul(t1[0:mr, :], rzs[0:mr, CW:2 * CW], t1[0:mr, :])
                        nc.vector.tensor_add(h_new[0:mr, mb, cols], nt[0:mr, :], t1[0:mr, :])


        # --- pooling + classifier ---
        h_fin = hA if (NSTEP * REP) % 2 == 0 else hB
        pooled = gpool.tile([P, 2, GPC], bf16, name="pooled", bufs=1)
        for g in range(GPC):
            a, b = int(goff[g]), int(goff[g] + gsize[g])
            nc.vector.tensor_reduce(pooled[:, :, g:g + 1], h_fin[:, :, a:b],
                                    axis=mybir.AxisListType.X,
                                    op=mybir.AluOpType.max)
        nc.vector.tensor_scalar_max(pooled[:], pooled[:], 0.0)
        lg = psum_hn.tile([2, GPC], f32, name="lg", tag="hn")
        nc.tensor.matmul(out=lg[:], lhsT=wcls_t[:, 0, :], rhs=pooled[:, 0, :],
                         start=True, stop=False)
        nc.tensor.matmul(out=lg[:], lhsT=wcls_t[0:K1, 1, :], rhs=pooled[0:K1, 1, :],
                         start=False, stop=True)
        ot = gpool.tile([2, GPC], f32, name="ot", bufs=1)
        nc.scalar.activation(ot[:], lg[:], AF.Sigmoid, bias=bcls_t[:])
        nc.sync.dma_start(out_d[:], ot[:])

    nc.compile()
    return nc


def kernel(**inputs):
    import os
    from concourse.bass_utils import run_bass_kernel_spmd

    per_core, meta = _preprocess(inputs["x"], inputs["edge_index"], inputs["batch"])
    w = _prep_weights(inputs["ggnn_weight"], inputs["w_ih"], inputs["w_hh"],
                      inputs["b_ih"], inputs["b_hh"], inputs["cls_w"], inputs["cls_b"])
    aggz = np.zeros((meta["HSD"] + 16, DP), BF16)
    in_maps = [dict(hT0=pc["hT0"], gidx=pc["gidx"],
                    sidx=pc["sidx"], aggz=aggz, **w) for pc in per_core]
    nc = _build_program(meta)
    trace = bool(int(os.environ.get("KTRACE", "0")))
    res = run_bass_kernel_spmd(nc, in_maps, core_ids=list(range(NC)), trace=trace)
    if trace:
        print(f"HW exec time: {res.exec_time_ns} ns")
        print("trace:", res.instructions_and_trace[1] if res.instructions_and_trace else None)
    out = np.zeros((NG, 2), np.float32)
    for c in range(NC):
        out[c * GPC:(c + 1) * GPC, :] = res.results[c]["out"].T
    return out

